# revision 4
# baseline (speedup 1.0000x reference)
"""Distributed Trainium2 Bass kernel for causal multi-head attention with RoPE.

Reference computation (B=2, S=2048, E=1024, H=16, D=64, fp32):
    q = rope((x @ Wq.T).heads); k = rope((x @ Wk.T).heads); v = (x @ Wv.T).heads
    out = softmax(mask(q k^T / sqrt(E))) v  -> concat heads -> @ Wo.T

Sharding (8 NeuronCores): data parallel over B (2 groups of 4 cores),
tensor parallel over heads within each group (4 heads per core).
Each core computes QKV for its 4 heads, flash-style causal attention,
normalized attention output transposed (d x s). AllGather (4-rank groups)
concatenates the per-head attention outputs, then every core computes a
256-column slice of the final Wo projection. No reduction collective needed.

Host-side prep (per-core input shards):
  - x fed transposed (E,S) in bf16.
  - Wq/Wk rows permuted per head to de-interleave RoPE pairs (even dims
    first, odd dims second) so RoPE becomes the rotate-half form.
  - cos/sin tables and the 32-row swap matrix are precomputed constants.
"""

import sys

sys.path.insert(0, "/opt/trn_rl_repo")

import numpy as np
import ml_dtypes

import concourse.bass as bass
import concourse.bacc as bacc
import concourse.mybir as mybir
import concourse.tile as tile
from concourse import bass_utils

B, S, E, H, D = 2, 2048, 1024, 16, 64
NCORES = 8
TP = 4                 # tensor-parallel group size
HPC = H // TP          # heads per core = 4
DQ = HPC * D           # per-core projection width = 256
ATTN_SCALE = 1.0 / float(np.sqrt(E))

FP32 = mybir.dt.float32
BF16 = mybir.dt.bfloat16

SQT = 512              # sq tile (free dim of S^T tiles)
SKB = 128              # sk block (partition dim of S^T tiles)
NSQT = S // SQT        # 4
NST16 = S // 128       # 16
NE = E // 128          # 8 contraction steps

REPLICA_GROUPS = [[0, 1, 2, 3], [4, 5, 6, 7]]

_CACHE = {}
LAST_RESULT = None


def build_nc():
    nc = bacc.Bacc(None, target_bir_lowering=False)

    xT = nc.declare_dram_parameter("xT", [E, S], BF16, isOutput=False)
    wqT = nc.declare_dram_parameter("wqT", [E, DQ], BF16, isOutput=False)
    wkT = nc.declare_dram_parameter("wkT", [E, DQ], BF16, isOutput=False)
    wvT = nc.declare_dram_parameter("wvT", [E, DQ], BF16, isOutput=False)
    woT = nc.declare_dram_parameter("woT", [E, DQ], BF16, isOutput=False)
    cosd = nc.declare_dram_parameter("cos", [128, S], FP32, isOutput=False)
    sind = nc.declare_dram_parameter("sin", [128, S], FP32, isOutput=False)
    swapd = nc.declare_dram_parameter("swapmat", [128, 128], BF16, isOutput=False)
    out_ext = nc.declare_dram_parameter("out", [S, DQ], FP32, isOutput=True)

    with tile.TileContext(nc) as tc:
        with (
            tc.tile_pool(name="dram", bufs=1, space="DRAM") as drampool,
            tc.tile_pool(name="const", bufs=1) as constpool,
            tc.tile_pool(name="qkv", bufs=1) as qkvpool,
        ):
            # ---- persistent SBUF tensors ----
            w_sb = {}
            for name, dram in (("wq", wqT), ("wk", wkT), ("wv", wvT), ("wo", woT)):
                t = constpool.tile([128, NE * DQ], BF16, tag=f"w_{name}", name=f"w_{name}")
                for j in range(NE):
                    nc.sync.dma_start(
                        out=t[:, j * DQ:(j + 1) * DQ],
                        in_=dram[j * 128:(j + 1) * 128, :],
                    )
                w_sb[name] = t
            cos_sb = constpool.tile([128, S], FP32, tag="cos")
            nc.sync.dma_start(out=cos_sb[:], in_=cosd[:])
            sin_sb = constpool.tile([128, S], FP32, tag="sin")
            nc.sync.dma_start(out=sin_sb[:], in_=sind[:])
            swap_sb = constpool.tile([128, 128], BF16, tag="swap")
            nc.sync.dma_start(out=swap_sb[:], in_=swapd[:])

            qt_sb = [constpool.tile([128, S], BF16, tag=f"qt{g}", name=f"qt{g}") for g in range(2)]
            kt_sb = [constpool.tile([128, S], BF16, tag=f"kt{g}", name=f"kt{g}") for g in range(2)]
            vaug = [
                constpool.tile([128, HPC * 65], BF16, tag=f"vaug{i}", name=f"vaug{i}")
                for i in range(NST16)
            ]
            attnT = [
                constpool.tile([64, S], BF16, tag=f"attn{h}", name=f"attn{h}") for h in range(HPC)
            ]

            # ---------------- Phase 1: QKV projections + RoPE ----------------
            with (
                tc.tile_pool(name="xt", bufs=1) as xtpool,
                tc.tile_pool(name="ps1", bufs=2, space="PSUM") as ps1pool,
                tc.tile_pool(name="ps2", bufs=2, space="PSUM") as ps2pool,
                tc.tile_pool(name="psv", bufs=2, space="PSUM") as psvpool,
                tc.tile_pool(name="ropetmp", bufs=3) as rtpool,
            ):
                xt = [xtpool.tile([128, S], BF16, tag=f"xT{j}", name=f"xT{j}") for j in range(NE)]
                for j in range(NE):
                    nc.sync.dma_start(
                        out=xt[j][:], in_=xT[j * 128:(j + 1) * 128, :]
                    )

                for g in range(2):
                    for st in range(NSQT):
                        sq = slice(st * SQT, (st + 1) * SQT)
                        for wname, dst in (("wq", qt_sb), ("wk", kt_sb)):
                            ps = ps1pool.tile([128, SQT], FP32, tag="ps")
                            for j in range(NE):
                                nc.tensor.matmul(
                                    ps[:],
                                    lhsT=w_sb[wname][
                                        :, j * DQ + g * 128: j * DQ + g * 128 + 128
                                    ],
                                    rhs=xt[j][:, sq],
                                    start=(j == 0),
                                    stop=(j == NE - 1),
                                )
                            raw = rtpool.tile([128, SQT], BF16, tag="raw")
                            nc.scalar.copy(raw[:], ps[:])
                            ps_sw = ps2pool.tile([128, SQT], FP32, tag="ps_sw")
                            nc.tensor.matmul(
                                ps_sw[:], lhsT=swap_sb[:], rhs=raw[:],
                                start=True, stop=True,
                            )
                            t1 = rtpool.tile([128, SQT], FP32, tag="t1")
                            nc.vector.tensor_mul(t1[:], ps_sw[:], sin_sb[:, sq])
                            t2 = rtpool.tile([128, SQT], FP32, tag="t2")
                            nc.vector.tensor_mul(t2[:], raw[:], cos_sb[:, sq])
                            nc.vector.tensor_add(dst[g][:, sq], t1[:], t2[:])

                # V projection (natural layout) + ones column augmentation
                for i in range(NST16):
                    psv = psvpool.tile([128, DQ], FP32, tag="psv")
                    for j in range(NE):
                        nc.tensor.matmul(
                            psv[:],
                            lhsT=xt[j][:, i * 128:(i + 1) * 128],
                            rhs=w_sb["wv"][:, j * DQ:(j + 1) * DQ],
                            start=(j == 0),
                            stop=(j == NE - 1),
                        )
                    nc.gpsimd.memset(vaug[i][:], 1.0)
                    for h in range(HPC):
                        nc.scalar.copy(
                            vaug[i][:, h * 65: h * 65 + 64],
                            psv[:, h * 64:(h + 1) * 64],
                        )

            # ---------------- Phase 2: causal flash attention ----------------
            with (
                tc.tile_pool(name="pss", bufs=2, space="PSUM") as psspool,
                tc.tile_pool(name="pso", bufs=2, space="PSUM") as psopool,
                tc.tile_pool(name="pt", bufs=3) as ptpool,
                tc.tile_pool(name="fin", bufs=2) as finpool,
            ):
                for g in range(2):
                    for p in range(2):
                        h = 2 * g + p
                        hp = slice(p * 64, (p + 1) * 64)
                        for st in range(NSQT):
                            sq = slice(st * SQT, (st + 1) * SQT)
                            pso = psopool.tile([65, SQT], FP32, tag=f"pso{p}")
                            nblk = (st + 1) * (SQT // SKB)
                            for kb in range(nblk):
                                pss = psspool.tile([SKB, SQT], FP32, tag=f"pss{p}")
                                nc.tensor.matmul(
                                    pss[:],
                                    lhsT=kt_sb[g][hp, kb * SKB:(kb + 1) * SKB],
                                    rhs=qt_sb[g][hp, sq],
                                    start=True,
                                    stop=True,
                                )
                                pt = ptpool.tile([SKB, SQT], BF16, tag=f"pt{p}")
                                nc.scalar.activation(
                                    pt[:], pss[:],
                                    mybir.ActivationFunctionType.Exp,
                                    scale=ATTN_SCALE,
                                )
                                if kb * SKB >= st * SQT:
                                    # diagonal block: zero entries with sq < sk
                                    nc.gpsimd.affine_select(
                                        out=pt[:],
                                        in_=pt[:],
                                        compare_op=mybir.AluOpType.is_ge,
                                        fill=0.0,
                                        base=st * SQT - kb * SKB,
                                        channel_multiplier=-1,
                                        pattern=[[1, SQT]],
                                    )
                                nc.tensor.matmul(
                                    pso[:],
                                    lhsT=vaug[kb][:, h * 65:(h + 1) * 65],
                                    rhs=pt[:],
                                    start=(kb == 0),
                                    stop=(kb == nblk - 1),
                                )
                            linv = finpool.tile([1, SQT], FP32, tag=f"linv{p}")
                            nc.vector.reciprocal(linv[:], pso[64:65, :])
                            lbc = finpool.tile([64, SQT], FP32, tag=f"lbc{p}")
                            nc.gpsimd.partition_broadcast(lbc[:], linv[:])
                            nc.vector.tensor_mul(
                                attnT[h][:, sq], pso[0:64, :], lbc[:]
                            )

            # ---------------- Phase 3: AllGather + Wo projection ----------------
            ag_in = drampool.tile([DQ, S], BF16, tag="ag_in")
            ag_out = drampool.tile([E, S], BF16, tag="ag_out")
            for h in range(HPC):
                nc.sync.dma_start(
                    out=ag_in[h * 64:(h + 1) * 64, :], in_=attnT[h][:]
                )
            nc.gpsimd.collective_compute(
                "AllGather",
                mybir.AluOpType.bypass,
                ins=[ag_in.opt()],
                outs=[ag_out.opt()],
                replica_groups=REPLICA_GROUPS,
            )

            with (
                tc.tile_pool(name="gt", bufs=1) as gtpool,
                tc.tile_pool(name="psw", bufs=2, space="PSUM") as pswpool,
                tc.tile_pool(name="osb", bufs=3) as osbpool,
            ):
                gt = [gtpool.tile([128, S], BF16, tag=f"gt{j}", name=f"gt{j}") for j in range(NE)]
                for j in range(NE):
                    nc.sync.dma_start(
                        out=gt[j][:], in_=ag_out[j * 128:(j + 1) * 128, :]
                    )
                for i in range(NST16):
                    psw = pswpool.tile([128, DQ], FP32, tag="psw")
                    for j in range(NE):
                        nc.tensor.matmul(
                            psw[:],
                            lhsT=gt[j][:, i * 128:(i + 1) * 128],
                            rhs=w_sb["wo"][:, j * DQ:(j + 1) * DQ],
                            start=(j == 0),
                            stop=(j == NE - 1),
                        )
                    osb = osbpool.tile([128, DQ], FP32, tag="osb")
                    nc.vector.tensor_copy(osb[:], psw[:])
                    nc.sync.dma_start(
                        out=out_ext[i * 128:(i + 1) * 128, :], in_=osb[:]
                    )

    nc.finalize()
    return nc


def _host_tables():
    inv = 1.0 / (10000.0 ** (np.arange(0, D, 2, dtype=np.float64) / D))  # (32,)
    ang = np.arange(S, dtype=np.float64)[None, :] * inv[:, None]          # (32,S)
    cos32 = np.cos(ang)
    sin32 = np.sin(ang)
    cos = np.tile(cos32, (4, 1)).astype(np.float32)                       # (128,S)
    sin = np.concatenate([-sin32, sin32, -sin32, sin32], axis=0).astype(np.float32)
    swap = np.zeros((128, 128), np.float32)
    for k in range(128):
        blk = (k // 64) * 64
        swap[k, blk + ((k - blk) + 32) % 64] = 1.0
    return cos, sin, swap


def kernel(x, W_q, W_k, W_v, W_o):
    global LAST_RESULT
    if "nc" not in _CACHE:
        _CACHE["nc"] = build_nc()
    nc = _CACHE["nc"]

    bf = ml_dtypes.bfloat16
    perm = np.concatenate([np.arange(0, D, 2), np.arange(1, D, 2)])
    rowperm = (np.arange(H)[:, None] * D + perm[None, :]).reshape(-1)
    Wq_p = W_q[rowperm]
    Wk_p = W_k[rowperm]
    cos, sin, swap = _host_tables()
    swap_bf = swap.astype(bf)

    in_maps = []
    for c in range(NCORES):
        b, tp = c // TP, c % TP
        sl = slice(tp * DQ, (tp + 1) * DQ)
        in_maps.append({
            "xT": np.ascontiguousarray(x[b].T).astype(bf),
            "wqT": np.ascontiguousarray(Wq_p[sl].T).astype(bf),
            "wkT": np.ascontiguousarray(Wk_p[sl].T).astype(bf),
            "wvT": np.ascontiguousarray(W_v[sl].T).astype(bf),
            "woT": np.ascontiguousarray(W_o[sl].T).astype(bf),
            "cos": cos,
            "sin": sin,
            "swapmat": swap_bf,
        })

    import os
    res = bass_utils.run_bass_kernel_spmd(
        nc, in_maps, core_ids=list(range(NCORES)),
        tmpdir=os.environ.get("BASS_TMPDIR") or None,
    )
    LAST_RESULT = res
    out = np.empty((B, S, E), np.float32)
    for c in range(NCORES):
        b, tp = c // TP, c % TP
        out[b][:, tp * DQ:(tp + 1) * DQ] = np.asarray(
            res.results[c]["out"], dtype=np.float32
        )
    return out


# revision 5
# speedup vs baseline: 1.0405x; 1.0405x over previous
"""Distributed Trainium2 Bass kernel for causal multi-head attention with RoPE.

Reference computation (B=2, S=2048, E=1024, H=16, D=64, fp32):
    q = rope((x @ Wq.T).heads); k = rope((x @ Wk.T).heads); v = (x @ Wv.T).heads
    out = softmax(mask(q k^T / sqrt(E))) v  -> concat heads -> @ Wo.T

Sharding (8 NeuronCores): data parallel over B (2 groups of 4 cores),
tensor parallel over heads within each group (4 heads per core).
Each core computes QKV for its 4 heads, flash-style causal attention,
normalized attention output transposed (d x s). A chunked AllGather
(4-rank groups, one chunk per 512-column sq block) concatenates the
per-head attention outputs while later chunks still compute; every core
then computes a 256-column slice of the final Wo projection per chunk.

Host-side prep (per-core input shards):
  - x fed transposed (E,S) in bf16.
  - Wq/Wk rows permuted per head to de-interleave RoPE pairs (even dims
    first, odd dims second) so RoPE becomes the rotate-half form.
  - cos/sin tables and the 32-row swap matrix are precomputed constants.
"""

import os
import sys

sys.path.insert(0, "/opt/trn_rl_repo")

import numpy as np
import ml_dtypes

import concourse.bass as bass
import concourse.bacc as bacc
import concourse.mybir as mybir
import concourse.tile as tile
from concourse import bass_utils

B, S, E, H, D = 2, 2048, 1024, 16, 64
NCORES = 8
TP = 4                 # tensor-parallel group size
HPC = H // TP          # heads per core = 4
DQ = HPC * D           # per-core projection width = 256
ATTN_SCALE = 1.0 / float(np.sqrt(E))

FP32 = mybir.dt.float32
BF16 = mybir.dt.bfloat16

SQT = 512              # sq tile (free dim of S^T tiles)
SKB = 128              # sk block (partition dim of S^T tiles)
NSQT = S // SQT        # 4
NST16 = S // 128       # 16
NE = E // 128          # 8 contraction steps

REPLICA_GROUPS = [[0, 1, 2, 3], [4, 5, 6, 7]]

_CACHE = {}
LAST_RESULT = None


def build_nc():
    nc = bacc.Bacc(None, target_bir_lowering=False)

    xT = nc.declare_dram_parameter("xT", [E, S], BF16, isOutput=False)
    wqT = nc.declare_dram_parameter("wqT", [E, DQ], BF16, isOutput=False)
    wkT = nc.declare_dram_parameter("wkT", [E, DQ], BF16, isOutput=False)
    wvT = nc.declare_dram_parameter("wvT", [E, DQ], BF16, isOutput=False)
    woT = nc.declare_dram_parameter("woT", [E, DQ], BF16, isOutput=False)
    cosd = nc.declare_dram_parameter("cos", [128, S], FP32, isOutput=False)
    sind = nc.declare_dram_parameter("sin", [128, S], FP32, isOutput=False)
    swapd = nc.declare_dram_parameter("swapmat", [128, 128], BF16, isOutput=False)
    out_ext = nc.declare_dram_parameter("out", [S, DQ], FP32, isOutput=True)

    with tile.TileContext(nc) as tc:
        with (
            tc.tile_pool(name="dram", bufs=1, space="DRAM") as drampool,
            tc.tile_pool(name="const", bufs=1) as constpool,
        ):
            # ---- persistent SBUF tensors; DMA order gates pipeline start ----
            w_sb = {}
            for name in ("wq", "wk", "wv", "wo"):
                w_sb[name] = constpool.tile(
                    [128, NE * DQ], BF16, tag=f"w_{name}", name=f"w_{name}"
                )

            def load_w(name, dram):
                for j in range(NE):
                    nc.sync.dma_start(
                        out=w_sb[name][:, j * DQ:(j + 1) * DQ],
                        in_=dram[j * 128:(j + 1) * 128, :],
                    )

            cos_sb = constpool.tile([128, S], FP32, tag="cos")
            sin_sb = constpool.tile([128, S], FP32, tag="sin")
            swap_sb = constpool.tile([128, 128], BF16, tag="swap")

            qt_sb = [
                constpool.tile([128, S], BF16, tag=f"qt{g}", name=f"qt{g}")
                for g in range(2)
            ]
            kt_sb = [
                constpool.tile([128, S], BF16, tag=f"kt{g}", name=f"kt{g}")
                for g in range(2)
            ]
            vaug = [
                constpool.tile([128, HPC * 65], BF16, tag=f"vaug{i}", name=f"vaug{i}")
                for i in range(NST16)
            ]
            attnT = [
                constpool.tile([64, S], BF16, tag=f"attn{h}", name=f"attn{h}")
                for h in range(HPC)
            ]

            # ---------------- Phase 1: QKV projections + RoPE ----------------
            with (
                tc.tile_pool(name="xt", bufs=1) as xtpool,
                tc.tile_pool(name="ps1", bufs=2, space="PSUM") as ps1pool,
                tc.tile_pool(name="ps2", bufs=2, space="PSUM") as ps2pool,
                tc.tile_pool(name="psv", bufs=2, space="PSUM") as psvpool,
                tc.tile_pool(name="ropetmp", bufs=3) as rtpool,
            ):
                xt = [
                    xtpool.tile([128, S], BF16, tag=f"xT{j}", name=f"xT{j}")
                    for j in range(NE)
                ]
                # gate-critical loads first
                load_w("wq", wqT)
                for j in range(NE):
                    nc.sync.dma_start(out=xt[j][:], in_=xT[j * 128:(j + 1) * 128, :])
                load_w("wk", wkT)
                nc.sync.dma_start(out=swap_sb[:], in_=swapd[:])
                nc.sync.dma_start(out=cos_sb[:], in_=cosd[:])
                nc.sync.dma_start(out=sin_sb[:], in_=sind[:])
                load_w("wv", wvT)
                load_w("wo", woT)

                for g in range(2):
                    for st in range(NSQT):
                        sq = slice(st * SQT, (st + 1) * SQT)
                        for wname, dst in (("wq", qt_sb), ("wk", kt_sb)):
                            ps = ps1pool.tile([128, SQT], FP32, tag="ps")
                            for j in range(NE):
                                nc.tensor.matmul(
                                    ps[:],
                                    lhsT=w_sb[wname][
                                        :, j * DQ + g * 128: j * DQ + g * 128 + 128
                                    ],
                                    rhs=xt[j][:, sq],
                                    start=(j == 0),
                                    stop=(j == NE - 1),
                                )
                            raw = rtpool.tile([128, SQT], BF16, tag="raw")
                            nc.scalar.copy(raw[:], ps[:])
                            ps_sw = ps2pool.tile([128, SQT], FP32, tag="ps_sw")
                            nc.tensor.matmul(
                                ps_sw[:], lhsT=swap_sb[:], rhs=raw[:],
                                start=True, stop=True,
                            )
                            t1 = rtpool.tile([128, SQT], FP32, tag="t1")
                            nc.vector.tensor_mul(t1[:], ps_sw[:], sin_sb[:, sq])
                            t2 = rtpool.tile([128, SQT], FP32, tag="t2")
                            nc.vector.tensor_mul(t2[:], raw[:], cos_sb[:, sq])
                            nc.vector.tensor_add(dst[g][:, sq], t1[:], t2[:])

                # V projection (natural layout) + ones column augmentation
                for i in range(NST16):
                    psv = psvpool.tile([128, DQ], FP32, tag="psv")
                    for j in range(NE):
                        nc.tensor.matmul(
                            psv[:],
                            lhsT=xt[j][:, i * 128:(i + 1) * 128],
                            rhs=w_sb["wv"][:, j * DQ:(j + 1) * DQ],
                            start=(j == 0),
                            stop=(j == NE - 1),
                        )
                    nc.gpsimd.memset(vaug[i][:], 1.0)
                    for h in range(HPC):
                        nc.scalar.copy(
                            vaug[i][:, h * 65: h * 65 + 64],
                            psv[:, h * 64:(h + 1) * 64],
                        )

            # ------- Phase 2+3: causal attention, chunked AG, Wo -------
            with (
                tc.tile_pool(name="pss", bufs=2, space="PSUM") as psspool,
                tc.tile_pool(name="pso", bufs=1, space="PSUM") as psopool,
                tc.tile_pool(name="psw", bufs=2, space="PSUM") as pswpool,
                tc.tile_pool(name="pt", bufs=3) as ptpool,
                tc.tile_pool(name="fin", bufs=2) as finpool,
                tc.tile_pool(name="gt", bufs=2) as gtpool,
                tc.tile_pool(name="osb", bufs=3) as osbpool,
            ):
                for st in range(NSQT):
                    sq = slice(st * SQT, (st + 1) * SQT)
                    nblk = (st + 1) * (SQT // SKB)
                    for g in range(2):
                        pso = [
                            psopool.tile([65, SQT], FP32, tag=f"pso{p}",
                                         name=f"pso{p}_{g}_{st}")
                            for p in range(2)
                        ]
                        for kb in range(nblk):
                            pss = psspool.tile([SKB, 2 * SQT], FP32, tag="pss")
                            for p in range(2):
                                nc.tensor.matmul(
                                    pss[:, p * SQT:(p + 1) * SQT],
                                    lhsT=kt_sb[g][
                                        p * 64:(p + 1) * 64, kb * SKB:(kb + 1) * SKB
                                    ],
                                    rhs=qt_sb[g][p * 64:(p + 1) * 64, sq],
                                    start=True,
                                    stop=True,
                                )
                            pt = ptpool.tile([SKB, 2 * SQT], BF16, tag="pt")
                            nc.scalar.activation(
                                pt[:], pss[:],
                                mybir.ActivationFunctionType.Exp,
                                scale=ATTN_SCALE,
                            )
                            if kb * SKB >= st * SQT:
                                # diagonal block: zero entries with sq < sk
                                # (same mask for both parity halves)
                                nc.gpsimd.affine_select(
                                    out=pt[:],
                                    in_=pt[:],
                                    compare_op=mybir.AluOpType.is_ge,
                                    fill=0.0,
                                    base=st * SQT - kb * SKB,
                                    channel_multiplier=-1,
                                    pattern=[[0, 2], [1, SQT]],
                                )
                            for p in range(2):
                                h = 2 * g + p
                                nc.tensor.matmul(
                                    pso[p][:],
                                    lhsT=vaug[kb][:, h * 65:(h + 1) * 65],
                                    rhs=pt[:, p * SQT:(p + 1) * SQT],
                                    start=(kb == 0),
                                    stop=(kb == nblk - 1),
                                )
                        for p in range(2):
                            h = 2 * g + p
                            linv = finpool.tile([1, SQT], FP32, tag=f"linv{p}")
                            nc.vector.reciprocal(linv[:], pso[p][64:65, :])
                            lbc = finpool.tile([64, SQT], FP32, tag=f"lbc{p}")
                            nc.gpsimd.partition_broadcast(lbc[:], linv[:])
                            nc.vector.tensor_mul(
                                attnT[h][:, sq], pso[p][0:64, :], lbc[:]
                            )

                    # ---- AllGather this sq chunk; Wo on gathered rows ----
                    agin = drampool.tile(
                        [DQ, SQT], BF16, tag=f"agin{st}", name=f"agin{st}"
                    )
                    agout = drampool.tile(
                        [E, SQT], BF16, tag=f"agout{st}", name=f"agout{st}"
                    )
                    for h in range(HPC):
                        nc.sync.dma_start(
                            out=agin[h * 64:(h + 1) * 64, :], in_=attnT[h][:, sq]
                        )
                    nc.gpsimd.collective_compute(
                        "AllGather",
                        mybir.AluOpType.bypass,
                        ins=[agin.opt()],
                        outs=[agout.opt()],
                        replica_groups=REPLICA_GROUPS,
                    )
                    gt = []
                    for j in range(NE):
                        t = gtpool.tile(
                            [128, SQT], BF16, tag=f"gt{j}", name=f"gt{j}_{st}"
                        )
                        nc.sync.dma_start(
                            out=t[:], in_=agout[j * 128:(j + 1) * 128, :]
                        )
                        gt.append(t)
                    for i4 in range(SQT // 128):
                        i = st * (SQT // 128) + i4
                        psw = pswpool.tile([128, DQ], FP32, tag="psw")
                        for j in range(NE):
                            nc.tensor.matmul(
                                psw[:],
                                lhsT=gt[j][:, i4 * 128:(i4 + 1) * 128],
                                rhs=w_sb["wo"][:, j * DQ:(j + 1) * DQ],
                                start=(j == 0),
                                stop=(j == NE - 1),
                            )
                        osb = osbpool.tile([128, DQ], FP32, tag="osb")
                        nc.vector.tensor_copy(osb[:], psw[:])
                        nc.sync.dma_start(
                            out=out_ext[i * 128:(i + 1) * 128, :], in_=osb[:]
                        )

    nc.finalize()
    return nc


def _host_tables():
    inv = 1.0 / (10000.0 ** (np.arange(0, D, 2, dtype=np.float64) / D))  # (32,)
    ang = np.arange(S, dtype=np.float64)[None, :] * inv[:, None]          # (32,S)
    cos32 = np.cos(ang)
    sin32 = np.sin(ang)
    cos = np.tile(cos32, (4, 1)).astype(np.float32)                       # (128,S)
    sin = np.concatenate([-sin32, sin32, -sin32, sin32], axis=0).astype(np.float32)
    swap = np.zeros((128, 128), np.float32)
    for k in range(128):
        blk = (k // 64) * 64
        swap[k, blk + ((k - blk) + 32) % 64] = 1.0
    return cos, sin, swap


def kernel(x, W_q, W_k, W_v, W_o):
    global LAST_RESULT
    if "nc" not in _CACHE:
        _CACHE["nc"] = build_nc()
    nc = _CACHE["nc"]

    bf = ml_dtypes.bfloat16
    perm = np.concatenate([np.arange(0, D, 2), np.arange(1, D, 2)])
    rowperm = (np.arange(H)[:, None] * D + perm[None, :]).reshape(-1)
    Wq_p = W_q[rowperm]
    Wk_p = W_k[rowperm]
    cos, sin, swap = _host_tables()
    swap_bf = swap.astype(bf)

    in_maps = []
    for c in range(NCORES):
        b, tp = c // TP, c % TP
        sl = slice(tp * DQ, (tp + 1) * DQ)
        in_maps.append({
            "xT": np.ascontiguousarray(x[b].T).astype(bf),
            "wqT": np.ascontiguousarray(Wq_p[sl].T).astype(bf),
            "wkT": np.ascontiguousarray(Wk_p[sl].T).astype(bf),
            "wvT": np.ascontiguousarray(W_v[sl].T).astype(bf),
            "woT": np.ascontiguousarray(W_o[sl].T).astype(bf),
            "cos": cos,
            "sin": sin,
            "swapmat": swap_bf,
        })

    res = bass_utils.run_bass_kernel_spmd(
        nc, in_maps, core_ids=list(range(NCORES)),
        tmpdir=os.environ.get("BASS_TMPDIR") or None,
    )
    LAST_RESULT = res
    out = np.empty((B, S, E), np.float32)
    for c in range(NCORES):
        b, tp = c // TP, c % TP
        out[b][:, tp * DQ:(tp + 1) * DQ] = np.asarray(
            res.results[c]["out"], dtype=np.float32
        )
    return out


# revision 12
# speedup vs baseline: 1.2091x; 1.1620x over previous
"""Distributed Trainium2 Bass kernel for causal multi-head attention with RoPE.

Reference computation (B=2, S=2048, E=1024, H=16, D=64, fp32):
    q = rope((x @ Wq.T).heads); k = rope((x @ Wk.T).heads); v = (x @ Wv.T).heads
    out = softmax(mask(q k^T / sqrt(E))) v  -> concat heads -> @ Wo.T

Sharding (8 NeuronCores): data parallel over B (2 groups of 4 cores),
tensor parallel over heads within each group (4 heads per core).
Each core computes QKV for its 4 heads, flash-style causal attention,
normalized attention output transposed (d x s). A chunked AllGather
(4-rank groups, one chunk per 512-column sq block) concatenates the
per-head attention outputs while later chunks still compute; every core
then computes a 256-column slice of the final Wo projection per chunk.

Host-side prep (per-core input shards):
  - x fed transposed (E,S) in bf16.
  - Wq/Wk rows permuted per head to de-interleave RoPE pairs (even dims
    first, odd dims second) so RoPE becomes the rotate-half form.
  - cos/sin tables and the 32-row swap matrix are precomputed constants.
"""

import os
import sys

sys.path.insert(0, "/opt/trn_rl_repo")

import numpy as np
import ml_dtypes

import concourse.bass as bass
import concourse.bacc as bacc
import concourse.mybir as mybir
import concourse.tile as tile
from concourse import bass_utils

B, S, E, H, D = 2, 2048, 1024, 16, 64
NCORES = 8
TP = 4                 # tensor-parallel group size
HPC = H // TP          # heads per core = 4
DQ = HPC * D           # per-core projection width = 256
ATTN_SCALE = 1.0 / float(np.sqrt(E))

FP32 = mybir.dt.float32
BF16 = mybir.dt.bfloat16

SQT = 512              # sq tile (free dim of S^T tiles)
SKB = 128              # sk block (partition dim of S^T tiles)
NSQT = S // SQT        # 4
NST16 = S // 128       # 16
NE = E // 128          # 8 contraction steps

REPLICA_GROUPS = [[0, 1, 2, 3], [4, 5, 6, 7]]

_CACHE = {}
LAST_RESULT = None


def build_nc():
    nc = bacc.Bacc(None, target_bir_lowering=False)

    xT = nc.declare_dram_parameter("xT", [E, S], BF16, isOutput=False)
    wqT = nc.declare_dram_parameter("wqT", [E, DQ], BF16, isOutput=False)
    wkT = nc.declare_dram_parameter("wkT", [E, DQ], BF16, isOutput=False)
    wvT = nc.declare_dram_parameter("wvT", [E, DQ], BF16, isOutput=False)
    woT = nc.declare_dram_parameter("woT", [E, DQ], BF16, isOutput=False)
    cosd = nc.declare_dram_parameter("cos", [128, S], FP32, isOutput=False)
    sind = nc.declare_dram_parameter("sin", [128, S], FP32, isOutput=False)
    swapd = nc.declare_dram_parameter("swapmat", [128, 128], BF16, isOutput=False)
    out_ext = nc.declare_dram_parameter("out", [S, DQ], FP32, isOutput=True)

    with tile.TileContext(nc) as tc:
        with (
            tc.tile_pool(name="dram", bufs=1, space="DRAM") as drampool,
            tc.tile_pool(name="const", bufs=1) as constpool,
        ):
            # ---- persistent SBUF tensors; DMA order gates pipeline start ----
            w_sb = {}
            for name in ("wq", "wk", "wv", "wo"):
                w_sb[name] = constpool.tile(
                    [128, NE * DQ], BF16, tag=f"w_{name}", name=f"w_{name}"
                )

            def load_w(name, dram):
                # one multi-dim DMA: DRAM (j*128+p, w) -> SBUF (p, j*DQ+w)
                nc.sync.dma_start(
                    out=w_sb[name][:].rearrange("p (j w) -> p j w", j=NE),
                    in_=dram[:].rearrange("(j p) w -> p j w", j=NE),
                )

            cos_sb = constpool.tile([128, S], FP32, tag="cos")
            sin_sb = constpool.tile([128, S], FP32, tag="sin")
            swap_sb = constpool.tile([128, 128], BF16, tag="swap")

            qt_sb = [
                constpool.tile([128, S], BF16, tag=f"qt{g}", name=f"qt{g}")
                for g in range(2)
            ]
            kt_sb = [
                constpool.tile([128, S], BF16, tag=f"kt{g}", name=f"kt{g}")
                for g in range(2)
            ]
            vaug = [
                constpool.tile([128, HPC * 65], BF16, tag=f"vaug{i}", name=f"vaug{i}")
                for i in range(NST16)
            ]
            attnT = [
                constpool.tile([64, S], BF16, tag=f"attn{h}", name=f"attn{h}")
                for h in range(HPC)
            ]

            # ---------------- Phase 1: QKV projections + RoPE ----------------
            with (
                tc.tile_pool(name="xt", bufs=1) as xtpool,
                tc.tile_pool(name="ps1", bufs=2, space="PSUM") as ps1pool,
                tc.tile_pool(name="ps2", bufs=2, space="PSUM") as ps2pool,
                tc.tile_pool(name="psv", bufs=2, space="PSUM") as psvpool,
                tc.tile_pool(name="ropetmp", bufs=3) as rtpool,
            ):
                xt = [
                    xtpool.tile([128, S], BF16, tag=f"xT{j}", name=f"xT{j}")
                    for j in range(NE)
                ]
                # gate-critical loads first
                load_w("wq", wqT)
                for j in range(NE):
                    nc.sync.dma_start(out=xt[j][:], in_=xT[j * 128:(j + 1) * 128, :])
                load_w("wk", wkT)
                nc.sync.dma_start(out=swap_sb[:], in_=swapd[:])
                nc.sync.dma_start(out=cos_sb[:], in_=cosd[:])
                nc.sync.dma_start(out=sin_sb[:], in_=sind[:])
                load_w("wv", wvT)
                load_w("wo", woT)

                for g in range(2):
                    for st in range(NSQT):
                        sq = slice(st * SQT, (st + 1) * SQT)
                        for wname, dst in (("wq", qt_sb), ("wk", kt_sb)):
                            ps = ps1pool.tile([128, SQT], FP32, tag="ps")
                            for j in range(NE):
                                nc.tensor.matmul(
                                    ps[:],
                                    lhsT=w_sb[wname][
                                        :, j * DQ + g * 128: j * DQ + g * 128 + 128
                                    ],
                                    rhs=xt[j][:, sq],
                                    start=(j == 0),
                                    stop=(j == NE - 1),
                                )
                            raw = rtpool.tile([128, SQT], BF16, tag="raw")
                            nc.scalar.copy(raw[:], ps[:])
                            ps_sw = ps2pool.tile([128, SQT], FP32, tag="ps_sw")
                            nc.tensor.matmul(
                                ps_sw[:], lhsT=swap_sb[:], rhs=raw[:],
                                start=True, stop=True,
                            )
                            t1 = rtpool.tile([128, SQT], FP32, tag="t1")
                            nc.vector.tensor_mul(t1[:], ps_sw[:], sin_sb[:, sq])
                            t2 = rtpool.tile([128, SQT], FP32, tag="t2")
                            nc.vector.tensor_mul(t2[:], raw[:], cos_sb[:, sq])
                            nc.vector.tensor_add(dst[g][:, sq], t1[:], t2[:])

                # V projection (natural layout) + ones column augmentation
                for i in range(NST16):
                    psv = psvpool.tile([128, DQ], FP32, tag="psv")
                    for j in range(NE):
                        nc.tensor.matmul(
                            psv[:],
                            lhsT=xt[j][:, i * 128:(i + 1) * 128],
                            rhs=w_sb["wv"][:, j * DQ:(j + 1) * DQ],
                            start=(j == 0),
                            stop=(j == NE - 1),
                        )
                    nc.gpsimd.memset(vaug[i][:], 1.0)
                    # one strided copy drops V into the 4 per-head 65-wide
                    # slots, leaving column 64 of each slot at 1.0
                    nc.scalar.copy(
                        vaug[i][:, 0:HPC * 65].rearrange(
                            "p (h w) -> p h w", h=HPC
                        )[:, :, 0:64],
                        psv[:].rearrange("p (h w) -> p h w", h=HPC),
                    )

            # ------- Phase 2+3: causal attention, chunked AG, Wo -------
            with (
                tc.tile_pool(name="pss", bufs=2, space="PSUM") as psspool,
                tc.tile_pool(name="pso", bufs=1, space="PSUM") as psopool,
                tc.tile_pool(name="psw", bufs=2, space="PSUM") as pswpool,
                tc.tile_pool(name="pt", bufs=3) as ptpool,
                tc.tile_pool(name="fin", bufs=2) as finpool,
                tc.tile_pool(name="gt", bufs=2) as gtpool,
                tc.tile_pool(name="osb", bufs=3) as osbpool,
            ):
                def wo_block(st, gt):
                    for i4 in range(SQT // 128):
                        i = st * (SQT // 128) + i4
                        psw = pswpool.tile(
                            [128, DQ], FP32, tag="psw", name=f"psw{i}"
                        )
                        for j in range(NE):
                            nc.tensor.matmul(
                                psw[:],
                                lhsT=gt[j][:, i4 * 128:(i4 + 1) * 128],
                                rhs=w_sb["wo"][:, j * DQ:(j + 1) * DQ],
                                start=(j == 0),
                                stop=(j == NE - 1),
                            )
                        osb = osbpool.tile(
                            [128, DQ], FP32, tag="osb", name=f"osb{i}"
                        )
                        nc.vector.tensor_copy(osb[:], psw[:])
                        nc.sync.dma_start(
                            out=out_ext[i * 128:(i + 1) * 128, :], in_=osb[:]
                        )

                pending = None  # (st, gt tiles) awaiting Wo
                for st in range(NSQT):
                    sq = slice(st * SQT, (st + 1) * SQT)
                    nblk = (st + 1) * (SQT // SKB)
                    for g in range(2):
                        pso = [
                            psopool.tile([65, SQT], FP32, tag=f"pso{p}",
                                         name=f"pso{p}_{g}_{st}")
                            for p in range(2)
                        ]
                        for kb in range(nblk):
                            pss = psspool.tile([SKB, 2 * SQT], FP32, tag="pss")
                            for p in range(2):
                                nc.tensor.matmul(
                                    pss[:, p * SQT:(p + 1) * SQT],
                                    lhsT=kt_sb[g][
                                        p * 64:(p + 1) * 64, kb * SKB:(kb + 1) * SKB
                                    ],
                                    rhs=qt_sb[g][p * 64:(p + 1) * 64, sq],
                                    start=True,
                                    stop=True,
                                )
                            pt = ptpool.tile([SKB, 2 * SQT], BF16, tag="pt")
                            nc.scalar.activation(
                                pt[:], pss[:],
                                mybir.ActivationFunctionType.Exp,
                                scale=ATTN_SCALE,
                            )
                            if kb * SKB >= st * SQT:
                                # diagonal block: zero entries with sq < sk
                                # (same mask for both parity halves)
                                nc.gpsimd.affine_select(
                                    out=pt[:],
                                    in_=pt[:],
                                    compare_op=mybir.AluOpType.is_ge,
                                    fill=0.0,
                                    base=st * SQT - kb * SKB,
                                    channel_multiplier=-1,
                                    pattern=[[0, 2], [1, SQT]],
                                )
                            for p in range(2):
                                h = 2 * g + p
                                nc.tensor.matmul(
                                    pso[p][:],
                                    lhsT=vaug[kb][:, h * 65:(h + 1) * 65],
                                    rhs=pt[:, p * SQT:(p + 1) * SQT],
                                    start=(kb == 0),
                                    stop=(kb == nblk - 1),
                                )
                        for p in range(2):
                            h = 2 * g + p
                            linv = finpool.tile([1, SQT], FP32, tag=f"linv{p}")
                            nc.vector.reciprocal(linv[:], pso[p][64:65, :])
                            lbc = finpool.tile([64, SQT], FP32, tag=f"lbc{p}")
                            nc.gpsimd.partition_broadcast(lbc[:], linv[:])
                            nc.vector.tensor_mul(
                                attnT[h][:, sq], pso[p][0:64, :], lbc[:]
                            )

                    # ---- AllGather this sq chunk; Wo on gathered rows ----
                    agin = drampool.tile(
                        [DQ, SQT], BF16, tag=f"agin{st}", name=f"agin{st}"
                    )
                    agout = drampool.tile(
                        [E, SQT], BF16, tag=f"agout{st}", name=f"agout{st}"
                    )
                    for h in range(HPC):
                        nc.sync.dma_start(
                            out=agin[h * 64:(h + 1) * 64, :], in_=attnT[h][:, sq]
                        )
                    nc.gpsimd.collective_compute(
                        "AllGather",
                        mybir.AluOpType.bypass,
                        ins=[agin.opt()],
                        outs=[agout.opt()],
                        replica_groups=REPLICA_GROUPS,
                    )
                    # Wo for the PREVIOUS chunk goes into the engine queues
                    # here, so its gathered tiles are long since ready and the
                    # in-order PE queue never stalls on this chunk's AllGather.
                    if pending is not None:
                        wo_block(*pending)
                    gt = []
                    for j in range(NE):
                        t = gtpool.tile(
                            [128, SQT], BF16, tag=f"gt{j}", name=f"gt{j}_{st}"
                        )
                        nc.sync.dma_start(
                            out=t[:], in_=agout[j * 128:(j + 1) * 128, :]
                        )
                        gt.append(t)
                    pending = (st, gt)
                wo_block(*pending)

    nc.finalize()
    return nc


def _host_tables():
    inv = 1.0 / (10000.0 ** (np.arange(0, D, 2, dtype=np.float64) / D))  # (32,)
    ang = np.arange(S, dtype=np.float64)[None, :] * inv[:, None]          # (32,S)
    cos32 = np.cos(ang)
    sin32 = np.sin(ang)
    cos = np.tile(cos32, (4, 1)).astype(np.float32)                       # (128,S)
    sin = np.concatenate([-sin32, sin32, -sin32, sin32], axis=0).astype(np.float32)
    swap = np.zeros((128, 128), np.float32)
    for k in range(128):
        blk = (k // 64) * 64
        swap[k, blk + ((k - blk) + 32) % 64] = 1.0
    return cos, sin, swap


def kernel(x, W_q, W_k, W_v, W_o):
    global LAST_RESULT
    if "nc" not in _CACHE:
        _CACHE["nc"] = build_nc()
    nc = _CACHE["nc"]

    bf = ml_dtypes.bfloat16
    perm = np.concatenate([np.arange(0, D, 2), np.arange(1, D, 2)])
    rowperm = (np.arange(H)[:, None] * D + perm[None, :]).reshape(-1)
    Wq_p = W_q[rowperm]
    Wk_p = W_k[rowperm]
    cos, sin, swap = _host_tables()
    swap_bf = swap.astype(bf)

    in_maps = []
    for c in range(NCORES):
        b, tp = c // TP, c % TP
        sl = slice(tp * DQ, (tp + 1) * DQ)
        in_maps.append({
            "xT": np.ascontiguousarray(x[b].T).astype(bf),
            "wqT": np.ascontiguousarray(Wq_p[sl].T).astype(bf),
            "wkT": np.ascontiguousarray(Wk_p[sl].T).astype(bf),
            "wvT": np.ascontiguousarray(W_v[sl].T).astype(bf),
            "woT": np.ascontiguousarray(W_o[sl].T).astype(bf),
            "cos": cos,
            "sin": sin,
            "swapmat": swap_bf,
        })

    res = bass_utils.run_bass_kernel_spmd(
        nc, in_maps, core_ids=list(range(NCORES)),
        tmpdir=os.environ.get("BASS_TMPDIR") or None,
    )
    LAST_RESULT = res
    out = np.empty((B, S, E), np.float32)
    for c in range(NCORES):
        b, tp = c // TP, c % TP
        out[b][:, tp * DQ:(tp + 1) * DQ] = np.asarray(
            res.results[c]["out"], dtype=np.float32
        )
    return out


# revision 18
# speedup vs baseline: 1.2510x; 1.0347x over previous
"""Distributed Trainium2 Bass kernel for causal multi-head attention with RoPE.

Reference computation (B=2, S=2048, E=1024, H=16, D=64, fp32):
    q = rope((x @ Wq.T).heads); k = rope((x @ Wk.T).heads); v = (x @ Wv.T).heads
    out = softmax(mask(q k^T / sqrt(E))) v  -> concat heads -> @ Wo.T

Sharding (8 NeuronCores): data parallel over B (2 groups of 4 cores),
tensor parallel over heads within each group (4 heads per core).
Each core computes QKV for its 4 heads, flash-style causal attention,
normalized attention output transposed (d x s). A chunked AllGather
(4-rank groups, one chunk per 512-column sq block) concatenates the
per-head attention outputs while later chunks still compute; every core
then computes a 256-column slice of the final Wo projection per chunk.

Host-side prep (per-core input shards):
  - x fed transposed (E,S) in bf16.
  - Wq/Wk rows permuted per head to de-interleave RoPE pairs (even dims
    first, odd dims second) so RoPE becomes the rotate-half form.
  - cos/sin tables and the 32-row swap matrix are precomputed constants.
"""

import os
import sys

sys.path.insert(0, "/opt/trn_rl_repo")

import numpy as np
import ml_dtypes

import concourse.bass as bass
import concourse.bacc as bacc
import concourse.mybir as mybir
import concourse.tile as tile
from concourse import bass_utils

B, S, E, H, D = 2, 2048, 1024, 16, 64
NCORES = 8
TP = 4                 # tensor-parallel group size
HPC = H // TP          # heads per core = 4
DQ = HPC * D           # per-core projection width = 256
ATTN_SCALE = 1.0 / float(np.sqrt(E))

FP32 = mybir.dt.float32
BF16 = mybir.dt.bfloat16

SQT = 512              # sq tile (free dim of S^T tiles)
SKB = 128              # sk block (partition dim of S^T tiles)
NSQT = S // SQT        # 4
NST16 = S // 128       # 16
NE = E // 128          # 8 contraction steps

REPLICA_GROUPS = [[0, 1, 2, 3], [4, 5, 6, 7]]

_CACHE = {}
LAST_RESULT = None


def build_nc():
    nc = bacc.Bacc(None, target_bir_lowering=False)

    xT = nc.declare_dram_parameter("xT", [E, S], BF16, isOutput=False)
    wqT = nc.declare_dram_parameter("wqT", [E, DQ], BF16, isOutput=False)
    wkT = nc.declare_dram_parameter("wkT", [E, DQ], BF16, isOutput=False)
    wvT = nc.declare_dram_parameter("wvT", [E, DQ], BF16, isOutput=False)
    woT = nc.declare_dram_parameter("woT", [E, DQ], BF16, isOutput=False)
    cosd = nc.declare_dram_parameter("cos", [128, S], FP32, isOutput=False)
    sind = nc.declare_dram_parameter("sin", [128, S], FP32, isOutput=False)
    swapd = nc.declare_dram_parameter("swapmat", [128, 128], BF16, isOutput=False)
    out_ext = nc.declare_dram_parameter("out", [S, DQ], FP32, isOutput=True)

    with tile.TileContext(nc) as tc:
        with (
            tc.tile_pool(name="dram", bufs=1, space="DRAM") as drampool,
            tc.tile_pool(name="const", bufs=1) as constpool,
        ):
            # ---- persistent SBUF tensors; DMA order gates pipeline start ----
            w_sb = {}
            for name in ("wq", "wk", "wv", "wo"):
                w_sb[name] = constpool.tile(
                    [128, NE * DQ], BF16, tag=f"w_{name}", name=f"w_{name}"
                )

            def load_w(name, dram):
                for j in range(NE):
                    nc.sync.dma_start(
                        out=w_sb[name][:, j * DQ:(j + 1) * DQ],
                        in_=dram[j * 128:(j + 1) * 128, :],
                    )

            cos_sb = constpool.tile([128, S], FP32, tag="cos")
            sin_sb = constpool.tile([128, S], FP32, tag="sin")
            swap_sb = constpool.tile([128, 128], BF16, tag="swap")

            qt_sb = [
                constpool.tile([128, S], BF16, tag=f"qt{g}", name=f"qt{g}")
                for g in range(2)
            ]
            kt_sb = [
                constpool.tile([128, S], BF16, tag=f"kt{g}", name=f"kt{g}")
                for g in range(2)
            ]
            vaug = [
                constpool.tile([128, HPC * 65], BF16, tag=f"vaug{i}", name=f"vaug{i}")
                for i in range(NST16)
            ]
            attnT = [
                constpool.tile([64, S], BF16, tag=f"attn{h}", name=f"attn{h}")
                for h in range(HPC)
            ]

            # ---------------- Phase 1: QKV projections + RoPE ----------------
            with (
                tc.tile_pool(name="xt", bufs=1) as xtpool,
                tc.tile_pool(name="ps1", bufs=2, space="PSUM") as ps1pool,
                tc.tile_pool(name="ps2", bufs=2, space="PSUM") as ps2pool,
                tc.tile_pool(name="psv", bufs=2, space="PSUM") as psvpool,
                tc.tile_pool(name="ropetmp", bufs=3) as rtpool,
            ):
                xt = [
                    xtpool.tile([128, S], BF16, tag=f"xT{j}", name=f"xT{j}")
                    for j in range(NE)
                ]
                # gate-critical loads first
                load_w("wq", wqT)
                for j in range(NE):
                    nc.sync.dma_start(out=xt[j][:], in_=xT[j * 128:(j + 1) * 128, :])
                load_w("wk", wkT)
                nc.sync.dma_start(out=swap_sb[:], in_=swapd[:])
                nc.sync.dma_start(out=cos_sb[:], in_=cosd[:])
                nc.sync.dma_start(out=sin_sb[:], in_=sind[:])
                load_w("wv", wvT)
                load_w("wo", woT)

                # V projection first: attention needs vaug tiles from kb=0,
                # so emitting V early keeps the attention pipeline unblocked.
                for i in range(NST16):
                    psv = psvpool.tile([128, DQ], FP32, tag="psv")
                    for j in range(NE):
                        nc.tensor.matmul(
                            psv[:],
                            lhsT=xt[j][:, i * 128:(i + 1) * 128],
                            rhs=w_sb["wv"][:, j * DQ:(j + 1) * DQ],
                            start=(j == 0),
                            stop=(j == NE - 1),
                        )
                    nc.gpsimd.memset(vaug[i][:], 1.0)
                    # one strided copy drops V into the 4 per-head 65-wide
                    # slots, leaving column 64 of each slot at 1.0
                    nc.scalar.copy(
                        vaug[i][:, 0:HPC * 65].rearrange(
                            "p (h w) -> p h w", h=HPC
                        )[:, :, 0:64],
                        psv[:].rearrange("p (h w) -> p h w", h=HPC),
                    )

                for g in range(2):
                    for st in range(NSQT):
                        sq = slice(st * SQT, (st + 1) * SQT)
                        for wname, dst in (("wq", qt_sb), ("wk", kt_sb)):
                            ps = ps1pool.tile([128, SQT], FP32, tag="ps")
                            for j in range(NE):
                                nc.tensor.matmul(
                                    ps[:],
                                    lhsT=w_sb[wname][
                                        :, j * DQ + g * 128: j * DQ + g * 128 + 128
                                    ],
                                    rhs=xt[j][:, sq],
                                    start=(j == 0),
                                    stop=(j == NE - 1),
                                )
                            raw = rtpool.tile([128, SQT], BF16, tag="raw")
                            nc.scalar.copy(raw[:], ps[:])
                            ps_sw = ps2pool.tile([128, SQT], FP32, tag="ps_sw")
                            nc.tensor.matmul(
                                ps_sw[:], lhsT=swap_sb[:], rhs=raw[:],
                                start=True, stop=True,
                            )
                            t1 = rtpool.tile([128, SQT], FP32, tag="t1")
                            nc.vector.tensor_mul(t1[:], ps_sw[:], sin_sb[:, sq])
                            t2 = rtpool.tile([128, SQT], FP32, tag="t2")
                            nc.vector.tensor_mul(t2[:], raw[:], cos_sb[:, sq])
                            nc.vector.tensor_add(dst[g][:, sq], t1[:], t2[:])

            # ------- Phase 2+3: causal attention, chunked AG, Wo -------
            with (
                tc.tile_pool(name="pss", bufs=2, space="PSUM") as psspool,
                tc.tile_pool(name="pso", bufs=1, space="PSUM") as psopool,
                tc.tile_pool(name="psw", bufs=2, space="PSUM") as pswpool,
                tc.tile_pool(name="pt", bufs=3) as ptpool,
                tc.tile_pool(name="fin", bufs=2) as finpool,
                tc.tile_pool(name="gt", bufs=2) as gtpool,
                tc.tile_pool(name="osb", bufs=3) as osbpool,
            ):
                # sq chunks of 512: parity halves of the S^T psum tile land in
                # separate PSUM banks (2KB each) — narrower chunks would make
                # the two concurrently-issued parity matmuls share one bank,
                # which is a fatal PSUM collision.
                CHUNKS = [(0, 512), (512, 512), (1024, 512), (1536, 512)]

                def wo_block(sq0, cw, gt):
                    for i4 in range(cw // 128):
                        r0 = sq0 + i4 * 128
                        psw = pswpool.tile(
                            [128, DQ], FP32, tag="psw", name=f"psw{r0}"
                        )
                        for j in range(NE):
                            nc.tensor.matmul(
                                psw[:],
                                lhsT=gt[j][:, i4 * 128:(i4 + 1) * 128],
                                rhs=w_sb["wo"][:, j * DQ:(j + 1) * DQ],
                                start=(j == 0),
                                stop=(j == NE - 1),
                            )
                        osb = osbpool.tile(
                            [128, DQ], FP32, tag="osb", name=f"osb{r0}"
                        )
                        nc.vector.tensor_copy(osb[:], psw[:])
                        nc.sync.dma_start(
                            out=out_ext[r0:r0 + 128, :], in_=osb[:]
                        )

                wo_queue = []  # (sq0, cw, gt tiles) awaiting Wo, 2-chunk lag
                for ci, (sq0, cw) in enumerate(CHUNKS):
                    sq = slice(sq0, sq0 + cw)
                    nblk = (sq0 + cw) // SKB
                    for g in range(2):
                        pso = [
                            psopool.tile([65, cw], FP32, tag=f"pso{p}",
                                         name=f"pso{p}_{g}_{ci}")
                            for p in range(2)
                        ]
                        for kb in range(nblk):
                            pss = psspool.tile([SKB, 2 * cw], FP32, tag="pss",
                                               name=f"pss_{g}_{ci}_{kb}")
                            for p in range(2):
                                nc.tensor.matmul(
                                    pss[:, p * cw:(p + 1) * cw],
                                    lhsT=kt_sb[g][
                                        p * 64:(p + 1) * 64, kb * SKB:(kb + 1) * SKB
                                    ],
                                    rhs=qt_sb[g][p * 64:(p + 1) * 64, sq],
                                    start=True,
                                    stop=True,
                                )
                            pt = ptpool.tile([SKB, 2 * cw], BF16, tag="pt",
                                             name=f"pt_{g}_{ci}_{kb}")
                            nc.scalar.activation(
                                pt[:], pss[:],
                                mybir.ActivationFunctionType.Exp,
                                scale=ATTN_SCALE,
                            )
                            if (kb + 1) * SKB > sq0:
                                # diagonal block: zero entries with sq < sk
                                # (same mask for both parity halves)
                                nc.gpsimd.affine_select(
                                    out=pt[:],
                                    in_=pt[:],
                                    compare_op=mybir.AluOpType.is_ge,
                                    fill=0.0,
                                    base=sq0 - kb * SKB,
                                    channel_multiplier=-1,
                                    pattern=[[0, 2], [1, cw]],
                                )
                            for p in range(2):
                                h = 2 * g + p
                                nc.tensor.matmul(
                                    pso[p][:],
                                    lhsT=vaug[kb][:, h * 65:(h + 1) * 65],
                                    rhs=pt[:, p * cw:(p + 1) * cw],
                                    start=(kb == 0),
                                    stop=(kb == nblk - 1),
                                )
                        # evacuate both pso tiles FIRST so their PSUM slots
                        # free ~0.5us after the last PV — the slow reciprocal
                        # chain then runs off the PE-critical path.
                        un = []
                        lrow = []
                        for p in range(2):
                            u = finpool.tile([64, cw], FP32, tag=f"un{p}",
                                             name=f"un{p}_{g}_{ci}")
                            nc.vector.tensor_copy(u[:], pso[p][0:64, :])
                            lr = finpool.tile([1, cw], FP32, tag=f"lrow{p}",
                                              name=f"lrow{p}_{g}_{ci}")
                            nc.vector.tensor_copy(lr[:], pso[p][64:65, :])
                            un.append(u)
                            lrow.append(lr)
                        for p in range(2):
                            h = 2 * g + p
                            linv = finpool.tile([1, cw], FP32, tag=f"linv{p}")
                            nc.vector.reciprocal(linv[:], lrow[p][:])
                            lbc = finpool.tile([64, cw], FP32, tag=f"lbc{p}")
                            nc.gpsimd.partition_broadcast(lbc[:], linv[:])
                            nc.vector.tensor_mul(
                                attnT[h][:, sq], un[p][:], lbc[:]
                            )

                    # ---- AllGather this sq chunk ----
                    agin = drampool.tile(
                        [DQ, cw], BF16, tag=f"agin{ci}", name=f"agin{ci}"
                    )
                    agout = drampool.tile(
                        [E, cw], BF16, tag=f"agout{ci}", name=f"agout{ci}"
                    )
                    for h in range(HPC):
                        nc.sync.dma_start(
                            out=agin[h * 64:(h + 1) * 64, :], in_=attnT[h][:, sq]
                        )
                    nc.gpsimd.collective_compute(
                        "AllGather",
                        mybir.AluOpType.bypass,
                        ins=[agin.opt()],
                        outs=[agout.opt()],
                        replica_groups=REPLICA_GROUPS,
                    )
                    # Wo runs with a TWO-chunk lag: by the time it enters the
                    # in-order PE queue its AllGather (plus the entry barrier)
                    # is long finished, so the queue never stalls on the CC
                    # stream.
                    if len(wo_queue) >= 2:
                        wo_block(*wo_queue.pop(0))
                    gt = []
                    for j in range(NE):
                        t = gtpool.tile(
                            [128, cw], BF16, tag=f"gt{j}", name=f"gt{j}_{ci}"
                        )
                        nc.sync.dma_start(
                            out=t[:], in_=agout[j * 128:(j + 1) * 128, :]
                        )
                        gt.append(t)
                    wo_queue.append((sq0, cw, gt))
                for args in wo_queue:
                    wo_block(*args)

    nc.finalize()
    return nc


def _host_tables():
    inv = 1.0 / (10000.0 ** (np.arange(0, D, 2, dtype=np.float64) / D))  # (32,)
    ang = np.arange(S, dtype=np.float64)[None, :] * inv[:, None]          # (32,S)
    cos32 = np.cos(ang)
    sin32 = np.sin(ang)
    cos = np.tile(cos32, (4, 1)).astype(np.float32)                       # (128,S)
    sin = np.concatenate([-sin32, sin32, -sin32, sin32], axis=0).astype(np.float32)
    swap = np.zeros((128, 128), np.float32)
    for k in range(128):
        blk = (k // 64) * 64
        swap[k, blk + ((k - blk) + 32) % 64] = 1.0
    return cos, sin, swap


def kernel(x, W_q, W_k, W_v, W_o):
    global LAST_RESULT
    if "nc" not in _CACHE:
        _CACHE["nc"] = build_nc()
    nc = _CACHE["nc"]

    bf = ml_dtypes.bfloat16
    perm = np.concatenate([np.arange(0, D, 2), np.arange(1, D, 2)])
    rowperm = (np.arange(H)[:, None] * D + perm[None, :]).reshape(-1)
    Wq_p = W_q[rowperm]
    Wk_p = W_k[rowperm]
    cos, sin, swap = _host_tables()
    swap_bf = swap.astype(bf)

    in_maps = []
    for c in range(NCORES):
        b, tp = c // TP, c % TP
        sl = slice(tp * DQ, (tp + 1) * DQ)
        in_maps.append({
            "xT": np.ascontiguousarray(x[b].T).astype(bf),
            "wqT": np.ascontiguousarray(Wq_p[sl].T).astype(bf),
            "wkT": np.ascontiguousarray(Wk_p[sl].T).astype(bf),
            "wvT": np.ascontiguousarray(W_v[sl].T).astype(bf),
            "woT": np.ascontiguousarray(W_o[sl].T).astype(bf),
            "cos": cos,
            "sin": sin,
            "swapmat": swap_bf,
        })

    res = bass_utils.run_bass_kernel_spmd(
        nc, in_maps, core_ids=list(range(NCORES)),
        tmpdir=os.environ.get("BASS_TMPDIR") or None,
    )
    LAST_RESULT = res
    out = np.empty((B, S, E), np.float32)
    for c in range(NCORES):
        b, tp = c // TP, c % TP
        out[b][:, tp * DQ:(tp + 1) * DQ] = np.asarray(
            res.results[c]["out"], dtype=np.float32
        )
    return out
